# revision 2
# baseline (speedup 1.0000x reference)
"""NF4 dequantization kernel for Trainium2 (8 NeuronCores, tensor-parallel).

Computes: out[g*32+r, n] = nf4_poly(quants[g, r, n]) * scales[g, 0, n]
where nf4_poly is a fixed degree-5 polynomial and quants hold 4-bit codes
(0..15) stored as int32.

Strategy (v3 — "one-pass")
--------------------------
The baseline (v2) split the work as ACT Square + DVE custom-op + DVE
tensor_tensor; since custom DVE ops run at 1x (123 G elem/s) and bf16 TT
at 2x, the DVE was ~102 us busy per core — the real bottleneck.

v3 collapses the entire computation into ONE custom DVE op per tile:

    out = w * ((w^2 + A*w + C)^2 + D) * s~        (8 ALU stages, 1 uop)

- The stored int8 codes `w` are a host-side 16-entry remap of the 4-bit
  codes onto quintic-preimage bytes: the quintic family
  k*w*((w^2+Aw+C)^2+D) has only 4 free shape params (3 fit in the
  C0/C1/imm2 scalar slots + k folded into the scales), but the 16 byte
  VALUES are free parameters too — jointly optimizing bytes + params
  fits the reference NF4 quintic to 2.7e-3 relative RMS.
- Src1 = scales, pre-replicated x8 on the HOST into DRAM ([G, 8, NS]
  bf16, 4 MiB/core) so the op reads a flat stride-1 operand — no
  replicate pass on any engine, no cross-engine dependency at all.
- Output int8 fixed-point (factor 126.5 folded into the scales), halving
  store traffic; host decodes with one multiply.

Per-core budgets: DVE 8 ops x [128, 8192] at 1x = ~69 us (the only
compute); DMA 8 (q) + 4 (s~) + 8 (out) = 20 MiB = ~59 us at 358 GB/s,
fully overlapped. Loads ride the SP ring, stores the Pool/gpsimd ring.

Error budget: fit 2.7e-3 + bf16 scales ~1.1e-3 + int8 out ~8e-3
=> ~9e-3 total (gate 2e-2).
"""

import numpy as np

import concourse.bacc as bacc
import concourse.mybir as mybir
import concourse.tile as tile
import concourse.dve_ops as dve_ops
from concourse.dve_spec import Spec, Src0, Src1, C0, C1, C2, sq, lower, _has_src1
from concourse.dve_uop import DveOpSpec

# ---------------------------------------------------------------- constants
_NCORES = 8
_G, _GS, _N = 256, 32, 8192          # full input shape
_NS = _N // _NCORES                  # 1024 columns per core
_RS = 8                              # group-rows per tile

# Joint fit of E(w) = k*w*((w^2+A*w+C)^2 + D) over free int8 code bytes
# to the 16 reference-quintic values (rel RMS 2.70e-3, see module doc).
_LUT = np.array(
    [-128, -97, -76, -60, -46, -33, -18, -2, 18, 39, 58, 73, 86, 99, 112, 127],
    dtype=np.int8,
)
_KQ = 2.29117802e-12                 # k (folded into scales host-side)
_AQ = 3.08772372e+02                 # A -> s0
_CQ = -1.58641742e+04                # C -> s1
_DQ = 1.86943859e+09                 # D -> imm2

_OUT_I8 = True                       # int8 fixed-point output transport
_I8_SCALE = 126.5                    # stays < 127 after all roundings
_BUFS = (2, 4, 4)                    # s, q, o tile-pool depths
_STORE_ENG = "gpsimd"                # HWDGE ring for output stores


def _register_op(name, spec):
    """Append a custom DVE op to the concourse registry (idempotent)."""
    for op in dve_ops.OPS:
        if op.name == name:
            return op
    row = dve_ops._CUSTOM_DVE_ROW_BASE + len(dve_ops.OPS)
    assert row < 0x20, "custom DVE opcode rows exhausted"
    shas = {
        ver: DveOpSpec(
            name=name, opcode=row, uops=lower(spec, ver=ver), rd1_en=_has_src1(spec)
        ).sha(ver)
        for ver in ("v3", "v4")
    }
    op = dve_ops.DveOp(name, spec, subdim=False, uops_sha=shas)
    dve_ops.OPS.append(op)
    dve_ops.CUSTOM_DVE_SPECS[name] = spec
    dve_ops._SUB_OPCODE_FOR_NAME[name] = row
    return op


def _make_op():
    s = sq(Src0)
    q = (s + Src0 * C0) + C1
    f = sq(q) + C2
    return _register_op(
        "NF4_1PASS_ANT",
        Spec(
            body=(Src0 * f) * Src1,
            reference=lambda in0, in1, s0, s1, imm2: in0
            * ((in0 * in0 + s0 * in0 + s1) ** 2 + imm2)
            * in1,
        ),
    )


_NC_CACHE = {}


def _build_module(_repeat=1):
    """Build + compile the per-core Bass module (identical on all cores).

    `_repeat` re-runs the whole loop nest N times over the same data —
    used only by benchmarking to measure marginal per-work time."""
    if _repeat in _NC_CACHE:
        return _NC_CACHE[_repeat]

    op = _make_op()
    nc = bacc.Bacc(
        "TRN2",
        target_bir_lowering=False,
        debug=False,
        enable_asserts=False,
        num_devices=_NCORES,
    )
    q_d = nc.dram_tensor(
        "quants", [_G, _GS, _NS], mybir.dt.int8, kind="ExternalInput"
    ).ap()
    s_d = nc.dram_tensor(
        "scales", [_G, _RS, _NS], mybir.dt.bfloat16, kind="ExternalInput"
    ).ap()
    out_dt = mybir.dt.int8 if _OUT_I8 else mybir.dt.bfloat16
    o_d = nc.dram_tensor(
        "out", [_G, _GS, _NS], out_dt, kind="ExternalOutput"
    ).ap()

    _GB = 128                        # groups per partition block
    fd = _RS * _NS
    with tile.TileContext(nc) as tc:
        with (
            tc.tile_pool(name="s", bufs=_BUFS[0]) as s_pool,
            tc.tile_pool(name="q", bufs=_BUFS[1]) as q_pool,
            tc.tile_pool(name="o", bufs=_BUFS[2]) as o_pool,
        ):
            for gb in [g for g in range(_G // _GB) for _ in range(_repeat)]:
                gsl = slice(gb * _GB, (gb + 1) * _GB)
                st = s_pool.tile([_GB, fd], mybir.dt.bfloat16, tag="s")
                nc.sync.dma_start(
                    st[:].rearrange("p (r n) -> p r n", r=_RS), s_d[gsl, :, :]
                )
                for rc in range(_GS // _RS):
                    rsl = slice(rc * _RS, (rc + 1) * _RS)
                    qt = q_pool.tile([_GB, fd], mybir.dt.int8, tag="q")
                    nc.sync.dma_start(
                        qt[:].rearrange("p (r n) -> p r n", r=_RS),
                        q_d[gsl, rsl, :],
                    )
                    ot = o_pool.tile([_GB, fd], out_dt, tag="o")
                    nc.vector._custom_dve(
                        op, out=ot[:], in0=qt[:], in1=st[:],
                        s0=_AQ, s1=_CQ, imm2=_DQ,
                    )
                    getattr(nc, _STORE_ENG).dma_start(
                        o_d[gsl, rsl, :],
                        ot[:].rearrange("p (r n) -> p r n", r=_RS),
                    )

    nc.compile()
    _NC_CACHE[_repeat] = nc
    return nc


def _prep_per_core(quants, scales):
    """Host-side input prep shared by kernel() and the bench harness.

    Returns a list of 8 dicts: per-core DRAM input arrays."""
    import jax

    bf16 = jax.numpy.bfloat16.dtype
    w8 = _LUT[np.asarray(quants)]                    # 16-entry code remap
    kscale = _KQ * (_I8_SCALE if _OUT_I8 else 1.0)
    s_k = (np.asarray(scales)[:, 0, :] * np.float64(kscale)).astype(np.float32)
    s_rep = np.broadcast_to(s_k[:, None, :], (_G, _RS, _N)).astype(bf16)
    per_core = []
    for i in range(_NCORES):
        csl = slice(i * _NS, (i + 1) * _NS)
        per_core.append(
            {
                "quants": np.ascontiguousarray(w8[:, :, csl]),
                "scales": np.ascontiguousarray(s_rep[:, :, csl]),
            }
        )
    return per_core


def _get_runner():
    """Cached jitted 8-core runner (shard_map over the axon devices)."""
    if "runner" in _NC_CACHE:
        return _NC_CACHE["runner"]

    import jax
    from jax.sharding import Mesh, NamedSharding, PartitionSpec
    from jax.experimental.shard_map import shard_map
    from concourse.bass2jax import _bass_exec_p, install_neuronx_cc_hook

    nc = _build_module()
    install_neuronx_cc_hook()

    in_names, out_names, out_avals, zero_outs = [], [], [], []
    for alloc in nc.m.functions[0].allocations:
        if not isinstance(alloc, mybir.MemoryLocationSet):
            continue
        name = alloc.memorylocations[0].name
        if alloc.kind == "ExternalInput":
            in_names.append(name)
        elif alloc.kind == "ExternalOutput":
            shape = tuple(alloc.tensor_shape)
            dtype = mybir.dt.np(alloc.dtype)
            out_names.append(name)
            out_avals.append(jax.core.ShapedArray(shape, dtype))
            zero_outs.append(np.zeros(shape, dtype))

    def _body(*args):
        return tuple(
            _bass_exec_p.bind(
                *args,
                out_avals=tuple(out_avals),
                in_names=tuple(in_names + out_names),
                out_names=tuple(out_names),
                lowering_input_output_aliases=(),
                sim_require_finite=True,
                sim_require_nnan=True,
                nc=nc,
            )
        )

    devices = jax.devices()[:_NCORES]
    mesh = Mesh(np.asarray(devices), ("core",))
    n_all = len(in_names) + len(out_names)
    sharded = jax.jit(
        shard_map(
            _body,
            mesh=mesh,
            in_specs=(PartitionSpec("core"),) * n_all,
            out_specs=(PartitionSpec("core"),) * len(out_names),
            check_rep=False,
        ),
        keep_unused=True,
    )
    sharding = NamedSharding(mesh, PartitionSpec("core"))
    # output placeholders: written by the NEFF, never read back -> resident
    zeros_dev = [
        jax.device_put(
            np.zeros((_NCORES * z.shape[0], *z.shape[1:]), z.dtype), sharding
        )
        for z in zero_outs
    ]
    runner = (sharded, in_names, out_names, sharding, zeros_dev)
    _NC_CACHE["runner"] = runner
    return runner


def kernel(quants: np.ndarray, scales: np.ndarray, **_) -> np.ndarray:
    quants = np.asarray(quants)
    scales = np.asarray(scales)
    assert quants.shape == (_G, _GS, _N) and scales.shape == (_G, 1, _N)

    import jax

    sharded, in_names, out_names, sharding, zeros_dev = _get_runner()

    per_core = _prep_per_core(quants, scales)
    by_name = {
        name: [pc[name] for pc in per_core] for name in per_core[0]
    }
    by_name["partition_id"] = [
        np.array([[i]], dtype=np.uint32) for i in range(_NCORES)
    ]
    args = [
        jax.device_put(np.concatenate(by_name[name], axis=0), sharding)
        for name in in_names
    ]
    outs = sharded(*args, *zeros_dev)
    out = np.asarray(outs[out_names.index("out")])  # [8*256, 32, 1024]
    # reassemble: core-shards on axis 0 -> columns of the full matrix
    full = (
        out.reshape(_NCORES, _G * _GS, _NS)
        .transpose(1, 0, 2)
        .reshape(_G * _GS, _N)
        .astype(np.float32)
    )
    if _OUT_I8:
        # decode the fixed-point transport format (value = i8 / 126.5)
        full *= np.float32(1.0 / _I8_SCALE)
    return full


if __name__ == "__main__":
    rng = np.random.default_rng(0)
    q = rng.integers(0, 16, (_G, _GS, _N)).astype(np.int32)
    s = rng.random((_G, 1, _N)).astype(np.float32)
    out = kernel(quants=q, scales=s)
    print("out", out.shape, out.dtype, out[0, :4])


# revision 6
# speedup vs baseline: 1.0210x; 1.0210x over previous
"""NF4 dequantization kernel for Trainium2 (8 NeuronCores, tensor-parallel).

Computes: out[g*32+r, n] = nf4_poly(quants[g, r, n]) * scales[g, 0, n]
where nf4_poly is a fixed degree-5 polynomial and quants hold 4-bit codes
(0..15) stored as int32.

Strategy (v3 — "one-pass")
--------------------------
The baseline (v2) split the work as ACT Square + DVE custom-op + DVE
tensor_tensor; since custom DVE ops run at 1x (123 G elem/s) and bf16 TT
at 2x, the DVE was ~102 us busy per core — the real bottleneck.

v3 collapses the entire computation into ONE custom DVE op per tile:

    out = w * ((w^2 + A*w + C)^2 + D) * s~        (8 ALU stages, 1 uop)

- The stored int8 codes `w` are a host-side 16-entry remap of the 4-bit
  codes onto quintic-preimage bytes: the quintic family
  k*w*((w^2+Aw+C)^2+D) has only 4 free shape params (3 fit in the
  C0/C1/imm2 scalar slots + k folded into the scales), but the 16 byte
  VALUES are free parameters too — jointly optimizing bytes + params
  fits the reference NF4 quintic to 2.7e-3 relative RMS.
- Src1 = scales, pre-replicated x8 on the HOST into DRAM ([G, 8, NS]
  bf16, 4 MiB/core) so the op reads a flat stride-1 operand — no
  replicate pass on any engine, no cross-engine dependency at all.
- Output int8 fixed-point (factor 126.5 folded into the scales), halving
  store traffic; host decodes with one multiply.

Per-core budgets: DVE 8 ops x [128, 8192] at 1x = ~69 us (the only
compute); DMA 8 (q) + 4 (s~) + 8 (out) = 20 MiB = ~59 us at 358 GB/s,
fully overlapped. Loads ride the SP ring, stores the Pool/gpsimd ring.

Error budget: fit 2.7e-3 + bf16 scales ~1.1e-3 + int8 out ~8e-3
=> ~9e-3 total (gate 2e-2).
"""

import numpy as np

import concourse.bacc as bacc
import concourse.mybir as mybir
import concourse.tile as tile
import concourse.dve_ops as dve_ops
from concourse.dve_spec import Spec, Src0, Src1, C0, C1, C2, sq, lower, _has_src1
from concourse.dve_uop import DveOpSpec

# ---------------------------------------------------------------- constants
_NCORES = 8
_G, _GS, _N = 256, 32, 8192          # full input shape
_NS = _N // _NCORES                  # 1024 columns per core
_RS = 8                              # group-rows per tile

# Joint fit of E(w) = k*w*((w^2+A*w+C)^2 + D) over free int8 code bytes
# to the 16 reference-quintic values (rel RMS 2.70e-3, see module doc).
_LUT = np.array(
    [-128, -97, -76, -60, -46, -33, -18, -2, 18, 39, 58, 73, 86, 99, 112, 127],
    dtype=np.int8,
)
_KQ = 2.29117802e-12                 # k (folded into scales host-side)
_AQ = 3.08772372e+02                 # A -> s0
_CQ = -1.58641742e+04                # C -> s1
_DQ = 1.86943859e+09                 # D -> imm2

_OUT_I8 = True                       # int8 fixed-point output transport
_I8_SCALE = 126.5                    # stays < 127 after all roundings
_BUFS = (2, 4, 4)                    # s, q, o tile-pool depths
_STORE_ENG = "gpsimd"                # HWDGE ring for output stores
_SREP_ONCHIP = True                  # replicate scales x8 on the idle ACT
                                     # engine instead of loading them
                                     # pre-replicated from DRAM (-3.5 MiB)


def _register_op(name, spec):
    """Append a custom DVE op to the concourse registry (idempotent)."""
    for op in dve_ops.OPS:
        if op.name == name:
            return op
    row = dve_ops._CUSTOM_DVE_ROW_BASE + len(dve_ops.OPS)
    assert row < 0x20, "custom DVE opcode rows exhausted"
    shas = {
        ver: DveOpSpec(
            name=name, opcode=row, uops=lower(spec, ver=ver), rd1_en=_has_src1(spec)
        ).sha(ver)
        for ver in ("v3", "v4")
    }
    op = dve_ops.DveOp(name, spec, subdim=False, uops_sha=shas)
    dve_ops.OPS.append(op)
    dve_ops.CUSTOM_DVE_SPECS[name] = spec
    dve_ops._SUB_OPCODE_FOR_NAME[name] = row
    return op


def _make_op():
    s = sq(Src0)
    q = (s + Src0 * C0) + C1
    f = sq(q) + C2
    return _register_op(
        "NF4_1PASS_ANT",
        Spec(
            body=(Src0 * f) * Src1,
            reference=lambda in0, in1, s0, s1, imm2: in0
            * ((in0 * in0 + s0 * in0 + s1) ** 2 + imm2)
            * in1,
        ),
    )


_NC_CACHE = {}


def _build_module(_repeat=1):
    """Build + compile the per-core Bass module (identical on all cores).

    `_repeat` re-runs the whole loop nest N times over the same data —
    used only by benchmarking to measure marginal per-work time."""
    if _repeat in _NC_CACHE:
        return _NC_CACHE[_repeat]

    op = _make_op()
    nc = bacc.Bacc(
        "TRN2",
        target_bir_lowering=False,
        debug=False,
        enable_asserts=False,
        num_devices=_NCORES,
    )
    q_d = nc.dram_tensor(
        "quants", [_G, _GS, _NS], mybir.dt.int8, kind="ExternalInput"
    ).ap()
    s_shape = [_G, _NS] if _SREP_ONCHIP else [_G, _RS, _NS]
    s_d = nc.dram_tensor(
        "scales", s_shape, mybir.dt.bfloat16, kind="ExternalInput"
    ).ap()
    out_dt = mybir.dt.int8 if _OUT_I8 else mybir.dt.bfloat16
    o_d = nc.dram_tensor(
        "out", [_G, _GS, _NS], out_dt, kind="ExternalOutput"
    ).ap()

    _GB = 128                        # groups per partition block
    fd = _RS * _NS
    with tile.TileContext(nc) as tc:
        with (
            tc.tile_pool(name="sc", bufs=2) as sc_pool,
            tc.tile_pool(name="s", bufs=_BUFS[0]) as s_pool,
            tc.tile_pool(name="q", bufs=_BUFS[1]) as q_pool,
            tc.tile_pool(name="o", bufs=_BUFS[2]) as o_pool,
        ):
            for gb in [g for g in range(_G // _GB) for _ in range(_repeat)]:
                gsl = slice(gb * _GB, (gb + 1) * _GB)
                st = s_pool.tile([_GB, fd], mybir.dt.bfloat16, tag="s")
                if _SREP_ONCHIP:
                    s_sm = sc_pool.tile([_GB, _NS], mybir.dt.bfloat16, tag="sc")
                    nc.sync.dma_start(s_sm[:], s_d[gsl, :])
                    nc.scalar.activation(
                        out=st[:].rearrange("p (r n) -> p r n", r=_RS),
                        in_=s_sm[:, None, :].broadcast_to([_GB, _RS, _NS]),
                        func=mybir.ActivationFunctionType.Copy,
                    )
                else:
                    nc.sync.dma_start(
                        st[:].rearrange("p (r n) -> p r n", r=_RS),
                        s_d[gsl, :, :],
                    )
                for rc in range(_GS // _RS):
                    rsl = slice(rc * _RS, (rc + 1) * _RS)
                    qt = q_pool.tile([_GB, fd], mybir.dt.int8, tag="q")
                    nc.sync.dma_start(
                        qt[:].rearrange("p (r n) -> p r n", r=_RS),
                        q_d[gsl, rsl, :],
                    )
                    ot = o_pool.tile([_GB, fd], out_dt, tag="o")
                    nc.vector._custom_dve(
                        op, out=ot[:], in0=qt[:], in1=st[:],
                        s0=_AQ, s1=_CQ, imm2=_DQ,
                    )
                    getattr(nc, _STORE_ENG).dma_start(
                        o_d[gsl, rsl, :],
                        ot[:].rearrange("p (r n) -> p r n", r=_RS),
                    )

    nc.compile()
    _NC_CACHE[_repeat] = nc
    return nc


def _prep_per_core(quants, scales):
    """Host-side input prep shared by kernel() and the bench harness.

    Returns a list of 8 dicts: per-core DRAM input arrays."""
    import jax

    bf16 = jax.numpy.bfloat16.dtype
    w8 = _LUT[np.asarray(quants)]                    # 16-entry code remap
    kscale = _KQ * (_I8_SCALE if _OUT_I8 else 1.0)
    s_k = (np.asarray(scales)[:, 0, :] * np.float64(kscale)).astype(np.float32)
    if _SREP_ONCHIP:
        s_host = s_k.astype(bf16)                     # [G, N]
    else:
        s_host = np.broadcast_to(s_k[:, None, :], (_G, _RS, _N)).astype(bf16)
    per_core = []
    for i in range(_NCORES):
        csl = slice(i * _NS, (i + 1) * _NS)
        per_core.append(
            {
                "quants": np.ascontiguousarray(w8[:, :, csl]),
                "scales": np.ascontiguousarray(s_host[..., csl]),
            }
        )
    return per_core


def _get_runner():
    """Cached jitted 8-core runner (shard_map over the axon devices)."""
    if "runner" in _NC_CACHE:
        return _NC_CACHE["runner"]

    import jax
    from jax.sharding import Mesh, NamedSharding, PartitionSpec
    from jax.experimental.shard_map import shard_map
    from concourse.bass2jax import _bass_exec_p, install_neuronx_cc_hook

    nc = _build_module()
    install_neuronx_cc_hook()

    in_names, out_names, out_avals, zero_outs = [], [], [], []
    for alloc in nc.m.functions[0].allocations:
        if not isinstance(alloc, mybir.MemoryLocationSet):
            continue
        name = alloc.memorylocations[0].name
        if alloc.kind == "ExternalInput":
            in_names.append(name)
        elif alloc.kind == "ExternalOutput":
            shape = tuple(alloc.tensor_shape)
            dtype = mybir.dt.np(alloc.dtype)
            out_names.append(name)
            out_avals.append(jax.core.ShapedArray(shape, dtype))
            zero_outs.append(np.zeros(shape, dtype))

    def _body(*args):
        return tuple(
            _bass_exec_p.bind(
                *args,
                out_avals=tuple(out_avals),
                in_names=tuple(in_names + out_names),
                out_names=tuple(out_names),
                lowering_input_output_aliases=(),
                sim_require_finite=True,
                sim_require_nnan=True,
                nc=nc,
            )
        )

    devices = jax.devices()[:_NCORES]
    mesh = Mesh(np.asarray(devices), ("core",))
    n_all = len(in_names) + len(out_names)
    sharded = jax.jit(
        shard_map(
            _body,
            mesh=mesh,
            in_specs=(PartitionSpec("core"),) * n_all,
            out_specs=(PartitionSpec("core"),) * len(out_names),
            check_rep=False,
        ),
        keep_unused=True,
    )
    sharding = NamedSharding(mesh, PartitionSpec("core"))
    # output placeholders: written by the NEFF, never read back -> resident
    zeros_dev = [
        jax.device_put(
            np.zeros((_NCORES * z.shape[0], *z.shape[1:]), z.dtype), sharding
        )
        for z in zero_outs
    ]
    runner = (sharded, in_names, out_names, sharding, zeros_dev)
    _NC_CACHE["runner"] = runner
    return runner


def kernel(quants: np.ndarray, scales: np.ndarray, **_) -> np.ndarray:
    quants = np.asarray(quants)
    scales = np.asarray(scales)
    assert quants.shape == (_G, _GS, _N) and scales.shape == (_G, 1, _N)

    import jax

    sharded, in_names, out_names, sharding, zeros_dev = _get_runner()

    per_core = _prep_per_core(quants, scales)
    by_name = {
        name: [pc[name] for pc in per_core] for name in per_core[0]
    }
    by_name["partition_id"] = [
        np.array([[i]], dtype=np.uint32) for i in range(_NCORES)
    ]
    args = [
        jax.device_put(np.concatenate(by_name[name], axis=0), sharding)
        for name in in_names
    ]
    outs = sharded(*args, *zeros_dev)
    out = np.asarray(outs[out_names.index("out")])  # [8*256, 32, 1024]
    # reassemble: core-shards on axis 0 -> columns of the full matrix
    full = (
        out.reshape(_NCORES, _G * _GS, _NS)
        .transpose(1, 0, 2)
        .reshape(_G * _GS, _N)
        .astype(np.float32)
    )
    if _OUT_I8:
        # decode the fixed-point transport format (value = i8 / 126.5)
        full *= np.float32(1.0 / _I8_SCALE)
    return full


if __name__ == "__main__":
    rng = np.random.default_rng(0)
    q = rng.integers(0, 16, (_G, _GS, _N)).astype(np.int32)
    s = rng.random((_G, 1, _N)).astype(np.float32)
    out = kernel(quants=q, scales=s)
    print("out", out.shape, out.dtype, out[0, :4])
